# revision 22
# baseline (speedup 1.0000x reference)
"""Trainium2 Bass kernel for nn_MinimalRNNCell.

Reference math (fp32):
    z_t = W_in x_t + b_in
    u_t = sigmoid(Wg_h h_{t-1} + Wg_z z_t + b_g)
    h_t = u_t * h_{t-1} + (1-u_t) * z_t
    y_t = W_out h_t + b_out
    output = y[:, batch=-1, :]  -> [T, O]   (only batch element 63 matters!)

Strategy (fixed-point iteration + hardware prefix scan, s-substitution):
  * Only sample 63 of the batch affects the output -> compute just that one.
  * With m := h - z and s_j := m_j + Delta_{j+1} (Delta_j := z_{j-1} - z_j)
    the update becomes   s_j = u_j s_{j-1} + Delta_{j+1},
    a first-order linear recurrence with ITERATION-STATIC coefficients --
    exactly the DVE's tensor_tensor_scan (state = u*state + d) with data1
    fixed to the precomputed Delta.  The gate argument also collapses:
        Wg_h m_{j-1} + Wg_z z_j + Wg_h z_{j-1} + b_g
      = Wg_h s_{j-1} + (Wg_z + Wg_h) z_j + b_g = Wg_h s_{j-1} + Q_j
    so each fixed-point round is just TWO matmuls (folded Q from x, plus
    Wg_h s), one sigmoid, and one scan per column chunk.  Round 0 uses
    s ~ Delta (i.e. m ~ 0) as the initial estimate via the same code path.
    The u->h coupling is weak (|dsigma|<=1/4, Wg_h entries ~0.06): each
    round contracts the error ~10x; three rounds reach ~8e-4 rel err in
    all-fp16, far under the 2e-2 gate.  m is recovered once at the end
    (m_j = u_j s_{j-1}) for the output matmul.
  * 8 cores each own 512 contiguous timesteps; each chunk restarts from
    m=0 with a W=16-column warmup prefix (error ~0.5^16, negligible).
    No serial per-timestep loop anywhere.
  * Host-folded fp16 weights: q65 = [((Wg_z+Wg_h) W_in)^T ; (Wg_z+Wg_h)
    b_in + b_g], winp/winn = +-[W_in^T ; b_in] (Delta straight from x),
    wox65 = [(W_out W_in)^T ; W_out b_in + b_out] (folds z and all biases
    into the output matmul: y = x~^T wox65 + m^T W_out^T, no z tensor).
    The ones-row of x~ carries the biases (zeroed where global t < 0).
  * s is ping-pong buffered per round so the two 264-column chunks
    decouple; PSUM tiles ring through banks; sigmoid ACT table preloads
    under the input DMAs.  The serial critical path per round is just
    matmul -> sigmoid -> scan.
"""

import numpy as np

import concourse.bass as bass
import concourse.mybir as mybir
import concourse.tile as tile
from concourse import bacc
from concourse.bass_utils import run_bass_kernel_spmd

# problem constants (hardcoded per harness contract)
T, I, H, O = 4096, 64, 128, 64
NCORES = 8
TLOC = T // NCORES          # timesteps per core
W = 16                      # warmup columns per core chunk
NZ = 2 + W + TLOC           # x columns (1 leading for z_{j-1}, 1 trailing)
N = W + TLOC                # scan columns j = 1..N
CH = N // 2                 # column chunk (264)
NITER = 2                   # fixed-point rounds (round 0 seeds s ~ Delta)
NCRIT = 384                 # c16 cols in the first (critical) weight DMA

_C16_COLS = {
    "q65": (0, 128), "winp": (128, 128), "winn": (256, 128),
    "wghh": (384, 128), "woutT": (512, 64), "wox65": (576, 64),
}
NC16 = 640

FP32 = mybir.dt.float32
FP16 = mybir.dt.float16
AF = mybir.ActivationFunctionType
OP = mybir.AluOpType


def _build_program():
    nc = bacc.Bacc()

    xT = nc.dram_tensor("xT", [I + 1, NZ], FP16, kind="ExternalInput")
    c16 = nc.dram_tensor("c16", [128, NC16], FP16, kind="ExternalInput")
    # y laid out [partition, block*O] so each partition's DMA is one
    # contiguous 1KB descriptor; the host unshuffles to [TLOC, O]
    y = nc.dram_tensor("y", [128, (TLOC // 128) * O], FP32,
                       kind="ExternalOutput")

    with tile.TileContext(nc) as tc:
        with (
            tc.tile_pool(name="singles", bufs=1) as singles,
            tc.tile_pool(name="state", bufs=2) as state,
            tc.tile_pool(name="psum", bufs=6, space="PSUM") as psum,
            tc.tile_pool(name="psum_y", bufs=1, space="PSUM") as psum_y,
        ):
            # ---- sigmoid ACT table preload (~1.3us) under the input DMAs ----
            junk = singles.tile([128, 1], FP32)
            nc.vector.memset(junk, 0.0)
            junk_sig = singles.tile([128, 1], FP32)
            nc.scalar.activation(junk_sig, junk, AF.Sigmoid)

            # ---- input DMAs (SP + Pool queues; ACT stays on the table) ----
            x_sb = singles.tile([I + 1, NZ], FP16)
            c_sb = singles.tile([128, NC16], FP16)
            # critical-path loads (prologue weights, first x half) ride the
            # SP queue, which issues DMAs earliest; the rest goes to Pool.
            # Splitting also means one straggler packet can't stall the start.
            XSPL = CH + 3
            nc.sync.dma_start(out=c_sb[:, 0:NCRIT], in_=c16[:, 0:NCRIT])
            nc.sync.dma_start(out=x_sb[:, 0:XSPL], in_=xT[:, 0:XSPL])
            nc.sync.dma_start(out=x_sb[:, XSPL:NZ], in_=xT[:, XSPL:NZ])
            nc.gpsimd.dma_start(out=c_sb[:, NCRIT:NC16], in_=c16[:, NCRIT:NC16])

            def cs(nm, rows=128):
                c0, n_ = _C16_COLS[nm]
                return c_sb[0:rows, c0:c0 + n_]

            q65 = cs("q65", I + 1)
            winp = cs("winp", I + 1)
            winn = cs("winn", I + 1)
            wghh = cs("wghh")
            woutT = cs("woutT")
            wox65 = cs("wox65", I + 1)

            # ---- Delta_j = z_{j-1} - z_j for j=1..N+1 -> d16 col j-1
            # (PE -> PSUM, DVE downcast to fp16 SBUF); emitted per column
            # chunk, interleaved with round 0, so chunk 0's whole
            # matmul -> cast -> sigmoid -> scan chain starts ASAP ----
            d16 = singles.tile([H, N + 1], FP16)
            sbuf = []
            for mi in range(2):
                sb = singles.tile([H, N + 1], FP16, name=f"s16_{mi}")
                sbuf.append(sb)
            ufin = [None, None]

            def delta_chunk(ci):
                c0, cn = (0, CH + 1) if ci == 0 else (CH + 1, CH)
                ps_d = psum.tile([H, CH + 1], FP32, tag="ps")
                nc.tensor.matmul(ps_d[:, 0:cn], winp, x_sb[:, c0:c0 + cn],
                                 start=True, stop=False)
                nc.tensor.matmul(ps_d[:, 0:cn], winn,
                                 x_sb[:, 1 + c0:1 + c0 + cn],
                                 start=False, stop=True)
                nc.vector.tensor_copy(d16[:, c0:c0 + cn], ps_d[:, 0:cn])
                if ci == 0:
                    # col 0 of both s buffers is the static s_0 = Delta_1
                    nc.vector.tensor_copy(sbuf[0][:, 0:1], d16[:, 0:1])
                    nc.vector.tensor_copy(sbuf[1][:, 0:1], d16[:, 0:1])

            def round_chunk(it, ci):
                scur = sbuf[it % 2]
                sprev = d16 if it == 0 else sbuf[(it + 1) % 2]
                c0 = ci * CH
                ps_u = psum.tile([H, CH], FP32, tag="ps")
                nc.tensor.matmul(ps_u, q65, x_sb[:, 1 + c0:1 + c0 + CH],
                                 start=True, stop=False)
                nc.tensor.matmul(ps_u, wghh, sprev[:, c0:c0 + CH],
                                 start=False, stop=True)
                u16 = state.tile([H, CH], FP16, tag=f"u{ci}")
                nc.scalar.activation(u16, ps_u, AF.Sigmoid)
                ufin[ci] = u16
                init = d16[:, 0:1] if ci == 0 else scur[:, c0:c0 + 1]
                nc.vector.tensor_tensor_scan(
                    scur[:, 1 + c0:1 + c0 + CH], u16,
                    d16[:, 1 + c0:1 + c0 + CH], init, OP.mult, OP.add)

            delta_chunk(0)
            round_chunk(0, 0)
            delta_chunk(1)
            round_chunk(0, 1)
            for it in range(1, NITER):
                round_chunk(it, 0)
                round_chunk(it, 1)

            # ---- m_j = u_j s_{j-1}; y = x~^T wox65 + m^T W_out^T ----
            sfin = sbuf[(NITER - 1) % 2]
            mfin = singles.tile([H, N + 1], FP16)
            ysb = singles.tile([128, TLOC // 128, O], FP32)
            ps_y = psum_y.tile([128, TLOC // 128, O], FP32, tag="ps_y")
            y_view = y.rearrange("p (b o) -> p b o", o=O)
            # block 0 (cols 17..145) only needs chunk 0's m; blocks 1-3 need
            # both chunks -> split 1 + 3 so the first DMA launches early
            for half, blocks in ((0, (0,)), (1, (1, 2, 3))):
                c0 = half * CH
                nc.vector.tensor_mul(mfin[:, 1 + c0:1 + c0 + CH],
                                     ufin[half], sfin[:, c0:c0 + CH])
                for b in blocks:
                    xs = slice(W + 1 + b * 128, W + 1 + (b + 1) * 128)
                    nc.tensor.matmul(ps_y[:, b, :], x_sb[:, xs], wox65,
                                     start=True, stop=False)
                    nc.tensor.matmul(ps_y[:, b, :], mfin[:, xs], woutT,
                                     start=False, stop=True)
                hb = slice(blocks[0], blocks[-1] + 1)
                nc.scalar.activation(ysb[:, hb, :], ps_y[:, hb, :], AF.Copy)
                nc.sync.dma_start(out=y_view[:, hb, :], in_=ysb[:, hb, :])

    nc.compile()
    return nc


_PROGRAM = None


def _get_program():
    global _PROGRAM
    if _PROGRAM is None:
        _PROGRAM = _build_program()
    return _PROGRAM


def _prepare_in_maps(inputs):
    x = np.ascontiguousarray(np.asarray(inputs["inputs"], dtype=np.float64)[63])
    W_in = np.asarray(inputs["W_in"], dtype=np.float64)
    b_in = np.asarray(inputs["b_in"], dtype=np.float64)
    W_g = np.asarray(inputs["W_g"], dtype=np.float64)
    b_g = np.asarray(inputs["b_g"], dtype=np.float64)
    W_out = np.asarray(inputs["W_out"], dtype=np.float64)
    b_out = np.asarray(inputs["b_out"], dtype=np.float64)

    Wg_h = W_g[:, :H]
    Wg_z = W_g[:, H:]

    c16 = np.zeros((128, NC16), np.float16)

    def put(name, v):
        c0, n_ = _C16_COLS[name]
        c16[:v.shape[0], c0:c0 + n_] = v.astype(np.float16)

    Wq = Wg_z + Wg_h
    put("q65", np.concatenate([(Wq @ W_in).T, (Wq @ b_in + b_g)[None, :]], 0))
    win65 = np.concatenate([W_in.T, b_in[None, :]], 0)
    put("winp", win65)
    put("winn", -win65)
    put("wghh", Wg_h.T)
    put("woutT", W_out.T)
    put("wox65", np.concatenate([(W_out @ W_in).T,
                                 (W_out @ b_in + b_out)[None, :]], 0))

    # x padded with W+1 leading and 1 trailing zero rows plus a ones-row
    # that carries the biases through the matmuls (zeroed where t < 0 or
    # t >= T)
    xpad = np.zeros((W + 1 + T + 1, I + 1), np.float16)
    xpad[W + 1:W + 1 + T, :I] = x.astype(np.float16)
    xpad[W + 1:W + 1 + T, I] = 1.0

    in_maps = []
    for k in range(NCORES):
        lo = k * TLOC
        xk = np.ascontiguousarray(xpad[lo:lo + NZ].T)
        in_maps.append({"xT": xk, "c16": c16})
    return in_maps


def _run(in_maps, **kwargs):
    nc = _get_program()
    return run_bass_kernel_spmd(nc, in_maps, list(range(NCORES)), **kwargs)


def _unshuffle(res):
    # per-core y arrives as [128, 4*O] (partition-contiguous); unshuffle to
    # [TLOC, O] with t = b*128 + p
    return np.concatenate(
        [res.results[k]["y"].reshape(128, TLOC // 128, O)
         .transpose(1, 0, 2).reshape(TLOC, O) for k in range(NCORES)],
        axis=0)


def kernel(**inputs):
    y = _unshuffle(_run(_prepare_in_maps(inputs)))
    return np.ascontiguousarray(y.astype(np.float32))


if __name__ == "__main__":
    d = np.load("/root/problem/inputs.npz")
    out = kernel(**{k: d[k] for k in d.files})
    exp = np.load("/root/problem/expected.npy")
    err = np.abs(out - exp).max()
    print("absmax err vs expected:", err, " rel:", err / np.abs(exp).max())


# revision 23
# speedup vs baseline: 1.2051x; 1.2051x over previous
"""Trainium2 Bass kernel for nn_MinimalRNNCell.

Reference math (fp32):
    z_t = W_in x_t + b_in
    u_t = sigmoid(Wg_h h_{t-1} + Wg_z z_t + b_g)
    h_t = u_t * h_{t-1} + (1-u_t) * z_t
    y_t = W_out h_t + b_out
    output = y[:, batch=-1, :]  -> [T, O]   (only batch element 63 matters!)

Strategy (fixed-point iteration + hardware prefix scan, s-substitution):
  * Only sample 63 of the batch affects the output -> compute just that one.
  * With m := h - z and s_j := m_j + Delta_{j+1} (Delta_j := z_{j-1} - z_j)
    the update becomes   s_j = u_j s_{j-1} + Delta_{j+1},
    a first-order linear recurrence with ITERATION-STATIC coefficients --
    exactly the DVE's tensor_tensor_scan (state = u*state + d), with data1
    read straight out of the PSUM tiles the Delta matmuls produced.  The
    gate argument collapses too:
        Wg_h m_{j-1} + Wg_z z_j + Wg_h z_{j-1} + b_g
      = Wg_h s_{j-1} + (Wg_z + Wg_h) z_j + b_g = Wg_h s_{j-1} + Q_j
    so each refinement round is TWO matmuls (folded Q from x, Wg_h s), one
    sigmoid, and one scan per column chunk.  Round 0 (the m ~ 0 estimate)
    instead uses the equivalent host-folded pair a65/b65 acting on x
    directly, so nothing in round 0 waits on the Delta pipeline.  The
    u->h coupling is weak (|dsigma|<=1/4, Wg_h entries ~0.06): each round
    contracts the error ~10x; two rounds reach ~7e-3 rel err in all-fp16
    vs the 2e-2 gate.  m is recovered once at the end (m_j = u_j s_{j-1})
    for the output matmul.
  * 8 cores each own 512 contiguous timesteps; each chunk restarts from
    m=0 with a W=16-column warmup prefix (error ~0.5^16, negligible).
    No serial per-timestep loop anywhere.
  * Host-folded fp16 weights (packed at their true row counts so the DMA
    ships no zero padding):
      a65 = [(Wg_z W_in)^T ; Wg_z b_in + b_g],  b65 = [(Wg_h W_in)^T ; Wg_h b_in]
      q65 = a65 + b65 folded = [((Wg_z+Wg_h) W_in)^T ; (Wg_z+Wg_h) b_in + b_g]
      winp/winn = +-[W_in^T ; b_in]  (Delta straight from x)
      wox65 = [(W_out W_in)^T ; W_out b_in + b_out]  (y = x~^T wox65 +
      m^T W_out^T -- z and every bias folded, no z tensor anywhere).
    The ones-row of x~ carries the biases (zeroed where global t < 0).
  * s is ping-pong buffered per round so the two 264-column chunks
    decouple; critical weights + first x half ride the earliest DMA queue;
    the sigmoid ACT table preloads under the DMAs; y leaves in a
    partition-contiguous layout the host unshuffles.
"""

import numpy as np

import concourse.bass as bass
import concourse.mybir as mybir
import concourse.tile as tile
from concourse import bacc
from concourse.bass_utils import run_bass_kernel_spmd

# problem constants (hardcoded per harness contract)
T, I, H, O = 4096, 64, 128, 64
NCORES = 8
TLOC = T // NCORES          # timesteps per core
W = 16                      # warmup columns per core chunk
NZ = 2 + W + TLOC           # x columns (1 leading for z_{j-1}, 1 trailing)
N = W + TLOC                # scan columns j = 1..N
CH = N // 2                 # column chunk (264)
NITER = 2                   # fixed-point rounds (round 0 seeds m ~ 0)

# 65-row blob: a65|b65|winp|winn|q65|wox65 ; 128-row blob: wghh|woutT
_C65_COLS = {
    "a65": (0, 128), "b65": (128, 128), "winp": (256, 128),
    "winn": (384, 128), "q65": (512, 128), "wox65": (640, 64),
}
NC65 = 704
NCRIT = 512                 # first weight DMA: a65..winn
_C128_COLS = {"wghh": (0, 128), "woutT": (128, 64)}
NC128 = 192

FP32 = mybir.dt.float32
FP16 = mybir.dt.float16
AF = mybir.ActivationFunctionType
OP = mybir.AluOpType


def _build_program():
    nc = bacc.Bacc()

    xT = nc.dram_tensor("xT", [I + 1, NZ], FP16, kind="ExternalInput")
    c65 = nc.dram_tensor("c65", [I + 1, NC65], FP16, kind="ExternalInput")
    c128 = nc.dram_tensor("c128", [128, NC128], FP16, kind="ExternalInput")
    # y laid out [partition, block*O] so each partition's DMA is one
    # contiguous 1KB descriptor; the host unshuffles to [TLOC, O]
    y = nc.dram_tensor("y", [128, (TLOC // 128) * O], FP32,
                       kind="ExternalOutput")

    with tile.TileContext(nc) as tc:
        with (
            tc.tile_pool(name="singles", bufs=1) as singles,
            tc.tile_pool(name="state", bufs=2) as state,
            tc.tile_pool(name="psum_d", bufs=2, space="PSUM") as psum_d,
            tc.tile_pool(name="psum", bufs=4, space="PSUM") as psum,
            tc.tile_pool(name="psum_y", bufs=1, space="PSUM") as psum_y,
        ):
            # ---- sigmoid ACT table preload (~1.4us) under the input DMAs ----
            junk = singles.tile([128, 1], FP32)
            nc.vector.memset(junk, 0.0)
            junk_sig = singles.tile([128, 1], FP32)
            nc.scalar.activation(junk_sig, junk, AF.Sigmoid)

            # ---- input DMAs: critical weights + first x half ride the SP
            # queue (earliest issue); the rest goes to the Pool queue ----
            x_sb = singles.tile([I + 1, NZ], FP16)
            c65_sb = singles.tile([I + 1, NC65], FP16)
            c128_sb = singles.tile([128, NC128], FP16)
            XSPL = CH + 3
            nc.sync.dma_start(out=c65_sb[:, 0:NCRIT], in_=c65[:, 0:NCRIT])
            nc.sync.dma_start(out=x_sb[:, 0:XSPL], in_=xT[:, 0:XSPL])
            nc.sync.dma_start(out=x_sb[:, XSPL:NZ], in_=xT[:, XSPL:NZ])
            nc.gpsimd.dma_start(out=c65_sb[:, NCRIT:NC65],
                                in_=c65[:, NCRIT:NC65])
            nc.gpsimd.dma_start(out=c128_sb, in_=c128[:, :])

            def c65s(nm):
                c0, n_ = _C65_COLS[nm]
                return c65_sb[:, c0:c0 + n_]

            a65, b65 = c65s("a65"), c65s("b65")
            winp, winn = c65s("winp"), c65s("winn")
            q65, wox65 = c65s("q65"), c65s("wox65")
            wghh = c128_sb[:, 0:128]
            woutT = c128_sb[:, 128:192]

            # s ping-pong buffers; col j = s_j, col 0 = s_0 = Delta_1
            sbuf = [singles.tile([H, N + 1], FP16, name=f"s16_{mi}")
                    for mi in range(2)]
            ufin = [None, None]
            ps_d = [None, None]

            # ---- per chunk: Delta_j -> PSUM (stays live; the scans read
            # their data1 straight from it), then the fixed-point rounds.
            # Chunk 0's whole chain is emitted first so it starts ASAP ----
            def delta_chunk(ci):
                c0, cn = (0, CH + 1) if ci == 0 else (CH + 1, CH)
                ps = psum_d.tile([H, CH + 1], FP32, tag="psd")
                nc.tensor.matmul(ps[:, 0:cn], winp, x_sb[:, c0:c0 + cn],
                                 start=True, stop=False)
                nc.tensor.matmul(ps[:, 0:cn], winn,
                                 x_sb[:, 1 + c0:1 + c0 + cn],
                                 start=False, stop=True)
                ps_d[ci] = ps
                if ci == 0:
                    # col 0 of both s buffers is the static s_0 = Delta_1
                    nc.vector.tensor_copy(sbuf[0][:, 0:1], ps[:, 0:1])
                    nc.vector.tensor_copy(sbuf[1][:, 0:1], ps[:, 0:1])

            def round_chunk(it, ci):
                scur = sbuf[it % 2]
                c0 = ci * CH
                ps_u = psum.tile([H, CH], FP32, tag="ps")
                if it == 0:   # m ~ 0 estimate, straight from x
                    nc.tensor.matmul(ps_u, a65, x_sb[:, 1 + c0:1 + c0 + CH],
                                     start=True, stop=False)
                    nc.tensor.matmul(ps_u, b65, x_sb[:, c0:c0 + CH],
                                     start=False, stop=True)
                else:
                    sprev = sbuf[(it + 1) % 2]
                    nc.tensor.matmul(ps_u, q65, x_sb[:, 1 + c0:1 + c0 + CH],
                                     start=True, stop=False)
                    nc.tensor.matmul(ps_u, wghh, sprev[:, c0:c0 + CH],
                                     start=False, stop=True)
                u16 = state.tile([H, CH], FP16, tag=f"u{ci}")
                nc.scalar.activation(u16, ps_u, AF.Sigmoid)
                ufin[ci] = u16
                # data1 = Delta_{j+1}: chunk 0 -> ps_d0 cols 1..264,
                # chunk 1 -> ps_d1 cols 0..263
                d1 = ps_d[0][:, 1:1 + CH] if ci == 0 else ps_d[1][:, 0:CH]
                init = ps_d[0][:, 0:1] if ci == 0 else scur[:, c0:c0 + 1]
                nc.vector.tensor_tensor_scan(
                    scur[:, 1 + c0:1 + c0 + CH], u16, d1, init,
                    OP.mult, OP.add)

            delta_chunk(0)
            round_chunk(0, 0)
            delta_chunk(1)
            round_chunk(0, 1)
            for it in range(1, NITER):
                round_chunk(it, 0)
                round_chunk(it, 1)

            # ---- m_j = u_j s_{j-1}; y = x~^T wox65 + m^T W_out^T ----
            sfin = sbuf[(NITER - 1) % 2]
            mfin = singles.tile([H, N + 1], FP16)
            ysb = singles.tile([128, TLOC // 128, O], FP32)
            ps_y = psum_y.tile([128, TLOC // 128, O], FP32, tag="ps_y")
            y_view = y.rearrange("p (b o) -> p b o", o=O)
            # block 0 (cols 17..145) only needs chunk 0's m; blocks 1-3 need
            # both chunks -> split 1 + 3 so the first DMA launches early
            for half, blocks in ((0, (0,)), (1, (1, 2, 3))):
                c0 = half * CH
                nc.vector.tensor_mul(mfin[:, 1 + c0:1 + c0 + CH],
                                     ufin[half], sfin[:, c0:c0 + CH])
                for b in blocks:
                    xs = slice(W + 1 + b * 128, W + 1 + (b + 1) * 128)
                    nc.tensor.matmul(ps_y[:, b, :], x_sb[:, xs], wox65,
                                     start=True, stop=False)
                    nc.tensor.matmul(ps_y[:, b, :], mfin[:, xs], woutT,
                                     start=False, stop=True)
                hb = slice(blocks[0], blocks[-1] + 1)
                nc.scalar.activation(ysb[:, hb, :], ps_y[:, hb, :], AF.Copy)
                nc.sync.dma_start(out=y_view[:, hb, :], in_=ysb[:, hb, :])

    nc.compile()
    return nc


_PROGRAM = None


def _get_program():
    global _PROGRAM
    if _PROGRAM is None:
        _PROGRAM = _build_program()
    return _PROGRAM


def _prepare_in_maps(inputs):
    x = np.ascontiguousarray(np.asarray(inputs["inputs"], dtype=np.float64)[63])
    W_in = np.asarray(inputs["W_in"], dtype=np.float64)
    b_in = np.asarray(inputs["b_in"], dtype=np.float64)
    W_g = np.asarray(inputs["W_g"], dtype=np.float64)
    b_g = np.asarray(inputs["b_g"], dtype=np.float64)
    W_out = np.asarray(inputs["W_out"], dtype=np.float64)
    b_out = np.asarray(inputs["b_out"], dtype=np.float64)

    Wg_h = W_g[:, :H]
    Wg_z = W_g[:, H:]

    c65 = np.zeros((I + 1, NC65), np.float16)
    c128 = np.zeros((128, NC128), np.float16)

    def put65(name, v):
        c0, n_ = _C65_COLS[name]
        c65[:, c0:c0 + n_] = v.astype(np.float16)

    put65("a65", np.concatenate([(Wg_z @ W_in).T,
                                 (Wg_z @ b_in + b_g)[None, :]], 0))
    put65("b65", np.concatenate([(Wg_h @ W_in).T, (Wg_h @ b_in)[None, :]], 0))
    win65 = np.concatenate([W_in.T, b_in[None, :]], 0)
    put65("winp", win65)
    put65("winn", -win65)
    Wq = Wg_z + Wg_h
    put65("q65", np.concatenate([(Wq @ W_in).T,
                                 (Wq @ b_in + b_g)[None, :]], 0))
    put65("wox65", np.concatenate([(W_out @ W_in).T,
                                   (W_out @ b_in + b_out)[None, :]], 0))
    c128[:, 0:128] = Wg_h.T.astype(np.float16)
    c128[:128, 128:192] = W_out.T.astype(np.float16)

    # x padded with W+1 leading and 1 trailing zero rows plus a ones-row
    # that carries the biases through the matmuls (zeroed where t < 0 or
    # t >= T)
    xpad = np.zeros((W + 1 + T + 1, I + 1), np.float16)
    xpad[W + 1:W + 1 + T, :I] = x.astype(np.float16)
    xpad[W + 1:W + 1 + T, I] = 1.0

    in_maps = []
    for k in range(NCORES):
        lo = k * TLOC
        xk = np.ascontiguousarray(xpad[lo:lo + NZ].T)
        in_maps.append({"xT": xk, "c65": c65, "c128": c128})
    return in_maps


def _run(in_maps, **kwargs):
    nc = _get_program()
    return run_bass_kernel_spmd(nc, in_maps, list(range(NCORES)), **kwargs)


def _unshuffle(res):
    # per-core y arrives as [128, 4*O] (partition-contiguous); unshuffle to
    # [TLOC, O] with t = b*128 + p
    return np.concatenate(
        [res.results[k]["y"].reshape(128, TLOC // 128, O)
         .transpose(1, 0, 2).reshape(TLOC, O) for k in range(NCORES)],
        axis=0)


def kernel(**inputs):
    y = _unshuffle(_run(_prepare_in_maps(inputs)))
    return np.ascontiguousarray(y.astype(np.float32))


if __name__ == "__main__":
    d = np.load("/root/problem/inputs.npz")
    out = kernel(**{k: d[k] for k in d.files})
    exp = np.load("/root/problem/expected.npy")
    err = np.abs(out - exp).max()
    print("absmax err vs expected:", err, " rel:", err / np.abs(exp).max())
